# revision 2
# baseline (speedup 1.0000x reference)
"""Grouped per-channel Linear + ReLU on 8 TRN2 NeuronCores.

Problem: out[b,c,e] = relu(sum_s x[b,s,c] * W[c,s,e] + bias[c,e])
  x: (256, 2048, 32) f32, W: (32, 2048, 2048) f32, bias: (32, 2048) f32
  out: (256, 32, 2048) f32

Sharding: expert/channel parallel — core i computes channels [4i, 4i+4).
Each core runs 4 independent GEMMs of (256x2048)@(2048x2048) with the
contraction dim S on SBUF partitions. x is pre-transposed on the host to
(C, S, B) so both matmul operands stream from DRAM with contiguous rows.

The bias is folded into the matmul accumulation group as an extra K=1
matmul (lhsT = ones(1,128), rhs = bias row), and ReLU happens during
PSUM->SBUF eviction on the Scalar engine.
"""

import os
import sys

for _p in ("/opt/trn_rl_repo", "/root/.axon_site/_ro/trn_rl_repo"):
    if os.path.isdir(_p) and _p not in sys.path:
        sys.path.insert(0, _p)

import numpy as np
import ml_dtypes

import concourse.bacc as bacc
import concourse.mybir as mybir
from concourse import tile
from concourse.bass_utils import run_bass_kernel_spmd

B, S, C, E = 256, 2048, 32, 2048
NCORES = 8
CPC = C // NCORES          # channels per core = 4
P = 128
KT = S // P                # 16 k-tiles
NBT = B // P               # 2 batch tiles
FREE = 512                 # matmul moving free dim (one PSUM bank of f32)
NET = E // FREE            # 4 e-tiles
NG = 2                     # e-tile groups; NET/NG psum banks live per group

# matmul dtype: "bfloat16" (fast, ~2e-3 rel), "float32r" (~1.5e-4 rel, 2x
# slower DMA-bound), "float32" (exact, 4x slower compute-bound)
MM_DTYPE = os.environ.get("KERNEL_MM_DTYPE", "bfloat16")

_DTYPES = {
    "bfloat16": (mybir.dt.bfloat16, ml_dtypes.bfloat16),
    "float32r": (mybir.dt.float32r, np.float32),
    "float32": (mybir.dt.float32, np.float32),
}

_nc_cache = {}


def _build(mm_dtype: str):
    mm_dt, _ = _DTYPES[mm_dtype]
    nc = bacc.Bacc(None, target_bir_lowering=False)
    xt = nc.dram_tensor("xt", [CPC, S, B], mm_dt, kind="ExternalInput")
    w = nc.dram_tensor("w", [CPC, S, E], mm_dt, kind="ExternalInput")
    bias = nc.dram_tensor("bias", [CPC, E], mm_dt, kind="ExternalInput")
    out = nc.dram_tensor("out", [B, CPC, E], mybir.dt.float32, kind="ExternalOutput")

    ET_PER_G = NET // NG

    with tile.TileContext(nc) as tc:
        with (
            tc.tile_pool(name="const", bufs=1) as const,
            tc.tile_pool(name="xpool", bufs=2) as xpool,
            tc.tile_pool(name="bpool", bufs=2) as bpool,
            tc.tile_pool(name="wpool", bufs=4 * ET_PER_G) as wpool,
            tc.tile_pool(name="opool", bufs=2 * ET_PER_G) as opool,
            tc.tile_pool(name="psum", bufs=2 * NBT * ET_PER_G, space="PSUM") as psum,
        ):
            ones = const.tile([1, P], mm_dt)
            nc.any.memset(ones[:], 1.0)
            zbias = const.tile([P, 1], mybir.dt.float32)
            nc.any.memset(zbias[:], 0.0)

            for c in range(CPC):
                # whole (S, B) slab for this channel: [p, k, b] in SBUF
                xsb = xpool.tile([P, KT, B], mm_dt)
                nc.sync.dma_start(
                    xsb[:], xt[c, :, :].rearrange("(k p) b -> p k b", p=P)
                )
                bsb = bpool.tile([1, E], mm_dt)
                nc.sync.dma_start(bsb[:], bias[c : c + 1, :])

                for g in range(NG):
                    ps = [
                        [
                            psum.tile([P, FREE], mybir.dt.float32, name="ps")
                            for _ in range(ET_PER_G)
                        ]
                        for _ in range(NBT)
                    ]
                    for k in range(KT):
                        wts = []
                        for j in range(ET_PER_G):
                            et = g * ET_PER_G + j
                            wt = wpool.tile([P, FREE], mm_dt)
                            nc.sync.dma_start(
                                wt[:],
                                w[c, k * P : (k + 1) * P, et * FREE : (et + 1) * FREE],
                            )
                            wts.append(wt)
                        for bt in range(NBT):
                            lhsT = xsb[:, k, bt * P : (bt + 1) * P]
                            for j in range(ET_PER_G):
                                nc.tensor.matmul(
                                    ps[bt][j][:],
                                    lhsT,
                                    wts[j][:],
                                    start=(k == 0),
                                    stop=False,
                                )
                    # bias row: psum += ones(1,128).T @ bias(1,FREE)
                    for bt in range(NBT):
                        for j in range(ET_PER_G):
                            et = g * ET_PER_G + j
                            nc.tensor.matmul(
                                ps[bt][j][:],
                                ones[0:1, :],
                                bsb[0:1, et * FREE : (et + 1) * FREE],
                                start=False,
                                stop=True,
                            )
                    # evict with fused ReLU on ScalarE, then DMA out
                    for bt in range(NBT):
                        for j in range(ET_PER_G):
                            et = g * ET_PER_G + j
                            ot = opool.tile([P, FREE], mybir.dt.float32)
                            nc.scalar.activation(
                                ot[:],
                                ps[bt][j][:],
                                mybir.ActivationFunctionType.Relu,
                                bias=zbias[:],
                            )
                            nc.sync.dma_start(
                                out[
                                    bt * P : (bt + 1) * P,
                                    c,
                                    et * FREE : (et + 1) * FREE,
                                ],
                                ot[:],
                            )
    nc.compile()
    return nc


def _get_nc(mm_dtype: str):
    if mm_dtype not in _nc_cache:
        _nc_cache[mm_dtype] = _build(mm_dtype)
    return _nc_cache[mm_dtype]


def _run(x, W, b, mm_dtype=None, **spmd_kwargs):
    mm_dtype = mm_dtype or MM_DTYPE
    _, np_dt = _DTYPES[mm_dtype]
    nc = _get_nc(mm_dtype)

    in_maps = []
    for i in range(NCORES):
        c0, c1 = i * CPC, (i + 1) * CPC
        xt_i = np.ascontiguousarray(
            x[:, :, c0:c1].transpose(2, 1, 0).astype(np_dt)
        )
        w_i = np.ascontiguousarray(W[c0:c1].astype(np_dt))
        b_i = np.ascontiguousarray(b[c0:c1].astype(np_dt))
        in_maps.append({"xt": xt_i, "w": w_i, "bias": b_i})

    res = run_bass_kernel_spmd(nc, in_maps, core_ids=list(range(NCORES)), **spmd_kwargs)
    out = np.concatenate([r["out"] for r in res.results], axis=1)
    return out, res


def kernel(x: np.ndarray, W: np.ndarray, b: np.ndarray) -> np.ndarray:
    out, _ = _run(x, W, b)
    return out


# revision 3
# speedup vs baseline: 1.4349x; 1.4349x over previous
"""Grouped per-channel Linear + ReLU on 8 TRN2 NeuronCores.

Problem: out[b,c,e] = relu(sum_s x[b,s,c] * W[c,s,e] + bias[c,e])
  x: (256, 2048, 32) f32, W: (32, 2048, 2048) f32, bias: (32, 2048) f32
  out: (256, 32, 2048) f32

Sharding: expert/channel parallel — core i computes channels [4i, 4i+4).
Each core runs 4 independent GEMMs of (256x2048)@(2048x2048) with the
contraction dim S on SBUF partitions. x is pre-transposed on the host to
(C, S, B) so both matmul operands stream from DRAM with contiguous rows.

Per channel: x slab (S,B) loaded once (1 MB DMA); W streamed in 2 MB
chunks (4 k-tiles x full E row) for DMA efficiency; all 8 PSUM banks hold
the (2 bt x 4 et) output block accumulating over 16 k-tiles. The bias is
folded in as an extra K=1 matmul (lhsT = ones(1,128), rhs = bias row).
ReLU happens during PSUM->SBUF eviction on the Scalar engine, and each
(bt, c) output row goes out as one 1 MB DMA.
"""

import os
import sys

for _p in ("/opt/trn_rl_repo", "/root/.axon_site/_ro/trn_rl_repo"):
    if os.path.isdir(_p) and _p not in sys.path:
        sys.path.insert(0, _p)

import numpy as np
import ml_dtypes

import concourse.bacc as bacc
import concourse.mybir as mybir
from concourse import tile
from concourse.bass_utils import run_bass_kernel_spmd

B, S, C, E = 256, 2048, 32, 2048
NCORES = 8
CPC = C // NCORES          # channels per core = 4
P = 128
KT = S // P                # 16 k-tiles
NBT = B // P               # 2 batch tiles
FREE = 512                 # matmul moving free dim (one PSUM bank of f32)
NET = E // FREE            # 4 e-tiles
KC = 4                     # k-tiles per W DMA chunk (2 MB chunks)

# matmul dtype: "bfloat16" (fast, ~2e-3 rel), "float32r" (~1.5e-4 rel,
# DMA-bound 2x slower), "float32" (exact, 4x slower compute-bound)
MM_DTYPE = os.environ.get("KERNEL_MM_DTYPE", "bfloat16")

_DTYPES = {
    "bfloat16": (mybir.dt.bfloat16, ml_dtypes.bfloat16),
    "float32r": (mybir.dt.float32r, np.float32),
    "float32": (mybir.dt.float32, np.float32),
}

_nc_cache = {}


def _build(mm_dtype: str):
    mm_dt, _ = _DTYPES[mm_dtype]
    nc = bacc.Bacc(None, target_bir_lowering=False)
    xt = nc.dram_tensor("xt", [CPC, S, B], mm_dt, kind="ExternalInput")
    w = nc.dram_tensor("w", [CPC, S, E], mm_dt, kind="ExternalInput")
    bias = nc.dram_tensor("bias", [CPC, E], mm_dt, kind="ExternalInput")
    out = nc.dram_tensor("out", [B, CPC, E], mybir.dt.float32, kind="ExternalOutput")

    with tile.TileContext(nc) as tc:
        with (
            tc.tile_pool(name="const", bufs=1) as const,
            tc.tile_pool(name="xpool", bufs=2) as xpool,
            tc.tile_pool(name="bpool", bufs=2) as bpool,
            tc.tile_pool(name="wpool", bufs=3) as wpool,
            tc.tile_pool(name="opool", bufs=3) as opool,
            tc.tile_pool(name="psum", bufs=NBT * NET, space="PSUM") as psum,
        ):
            ones = const.tile([1, P], mm_dt)
            nc.any.memset(ones[:], 1.0)
            zbias = const.tile([P, 1], mybir.dt.float32)
            nc.any.memset(zbias[:], 0.0)

            for c in range(CPC):
                # whole (S, B) slab for this channel: [p, k, b] in SBUF, 1 MB
                xsb = xpool.tile([P, KT, B], mm_dt)
                nc.sync.dma_start(
                    xsb[:], xt[c, :, :].rearrange("(k p) b -> p k b", p=P)
                )
                bsb = bpool.tile([1, E], mm_dt)
                nc.sync.dma_start(bsb[:], bias[c : c + 1, :])

                ps = [
                    [
                        psum.tile([P, FREE], mybir.dt.float32, name="ps")
                        for _ in range(NET)
                    ]
                    for _ in range(NBT)
                ]
                for kc in range(KT // KC):
                    # 2 MB W chunk: 4 k-tiles x full E row
                    wsb = wpool.tile([P, KC, E], mm_dt)
                    nc.sync.dma_start(
                        wsb[:],
                        w[c, kc * KC * P : (kc + 1) * KC * P, :].rearrange(
                            "(k p) e -> p k e", p=P
                        ),
                    )
                    for kk in range(KC):
                        k = kc * KC + kk
                        for bt in range(NBT):
                            lhsT = xsb[:, k, bt * P : (bt + 1) * P]
                            for et in range(NET):
                                nc.tensor.matmul(
                                    ps[bt][et][:],
                                    lhsT,
                                    wsb[:, kk, et * FREE : (et + 1) * FREE],
                                    start=(k == 0),
                                    stop=False,
                                )
                # bias row: psum += ones(1,128).T @ bias(1,FREE)
                for bt in range(NBT):
                    for et in range(NET):
                        nc.tensor.matmul(
                            ps[bt][et][:],
                            ones[0:1, :],
                            bsb[0:1, et * FREE : (et + 1) * FREE],
                            start=False,
                            stop=True,
                        )
                # evict with fused ReLU on ScalarE into one (128, E) row,
                # then one 1 MB DMA out per bt
                for bt in range(NBT):
                    ot = opool.tile([P, E], mybir.dt.float32)
                    for et in range(NET):
                        nc.scalar.activation(
                            ot[:, et * FREE : (et + 1) * FREE],
                            ps[bt][et][:],
                            mybir.ActivationFunctionType.Relu,
                            bias=zbias[:],
                        )
                    nc.sync.dma_start(out[bt * P : (bt + 1) * P, c, :], ot[:])
    nc.compile()
    return nc


def _get_nc(mm_dtype: str):
    if mm_dtype not in _nc_cache:
        _nc_cache[mm_dtype] = _build(mm_dtype)
    return _nc_cache[mm_dtype]


def _run(x, W, b, mm_dtype=None, **spmd_kwargs):
    mm_dtype = mm_dtype or MM_DTYPE
    _, np_dt = _DTYPES[mm_dtype]
    nc = _get_nc(mm_dtype)

    in_maps = []
    for i in range(NCORES):
        c0, c1 = i * CPC, (i + 1) * CPC
        xt_i = np.ascontiguousarray(
            x[:, :, c0:c1].transpose(2, 1, 0).astype(np_dt)
        )
        w_i = np.ascontiguousarray(W[c0:c1].astype(np_dt))
        b_i = np.ascontiguousarray(b[c0:c1].astype(np_dt))
        in_maps.append({"xt": xt_i, "w": w_i, "bias": b_i})

    res = run_bass_kernel_spmd(nc, in_maps, core_ids=list(range(NCORES)), **spmd_kwargs)
    out = np.concatenate([r["out"] for r in res.results], axis=1)
    return out, res


def kernel(x: np.ndarray, W: np.ndarray, b: np.ndarray) -> np.ndarray:
    out, _ = _run(x, W, b)
    return out


# revision 6
# speedup vs baseline: 1.4848x; 1.0348x over previous
"""Grouped per-channel Linear + ReLU on 8 TRN2 NeuronCores.

Problem: out[b,c,e] = relu(sum_s x[b,s,c] * W[c,s,e] + bias[c,e])
  x: (256, 2048, 32) f32, W: (32, 2048, 2048) f32, bias: (32, 2048) f32
  out: (256, 32, 2048) f32

Sharding: expert/channel parallel — core i computes channels [4i, 4i+4).
Each core runs 4 independent GEMMs of (256x2048)@(2048x2048) with the
contraction dim S on SBUF partitions. x is pre-transposed on the host to
(C, S, B) so both matmul operands stream from DRAM with contiguous rows.

Per channel: x slab (S,B) loaded once (1 MB DMA); W streamed in 2 MB
chunks (4 k-tiles x full E row) for DMA efficiency; all 8 PSUM banks hold
the (2 bt x 4 et) output block accumulating over 16 k-tiles. The bias is
folded in as an extra K=1 matmul (lhsT = ones(1,128), rhs = bias row).
ReLU happens during PSUM->SBUF eviction on the Scalar engine, and each
(bt, c) output row goes out as one 1 MB DMA.
"""

import os
import sys

for _p in ("/opt/trn_rl_repo", "/root/.axon_site/_ro/trn_rl_repo"):
    if os.path.isdir(_p) and _p not in sys.path:
        sys.path.insert(0, _p)

import numpy as np
import ml_dtypes

import concourse.bacc as bacc
import concourse.mybir as mybir
from concourse import tile
from concourse.bass_utils import run_bass_kernel_spmd

B, S, C, E = 256, 2048, 32, 2048
NCORES = 8
CPC = C // NCORES          # channels per core = 4
P = 128
KT = S // P                # 16 k-tiles
NBT = B // P               # 2 batch tiles
FREE = 512                 # matmul moving free dim (one PSUM bank of f32)
NET = E // FREE            # 4 e-tiles
KC = 4                     # k-tiles per W DMA chunk (2 MB chunks)

# matmul dtype: "bfloat16" (fast, ~2e-3 rel), "float32r" (~1.5e-4 rel,
# DMA-bound 2x slower), "float32" (exact, 4x slower compute-bound)
MM_DTYPE = os.environ.get("KERNEL_MM_DTYPE", "bfloat16")

_DTYPES = {
    "bfloat16": (mybir.dt.bfloat16, ml_dtypes.bfloat16),
    "float32r": (mybir.dt.float32r, np.float32),
    "float32": (mybir.dt.float32, np.float32),
}

_nc_cache = {}


def _build(mm_dtype: str):
    mm_dt, _ = _DTYPES[mm_dtype]
    nc = bacc.Bacc(None, target_bir_lowering=False)
    xt = nc.dram_tensor("xt", [CPC, S, B], mm_dt, kind="ExternalInput")
    w = nc.dram_tensor("w", [CPC, S, E], mm_dt, kind="ExternalInput")
    bias = nc.dram_tensor("bias", [CPC, E], mm_dt, kind="ExternalInput")
    out = nc.dram_tensor("out", [B, CPC, E], mybir.dt.float32, kind="ExternalOutput")

    with tile.TileContext(nc) as tc:
        with (
            tc.tile_pool(name="const", bufs=1) as const,
            tc.tile_pool(name="xpool", bufs=2) as xpool,
            tc.tile_pool(name="bpool", bufs=2) as bpool,
            tc.tile_pool(name="wpool", bufs=4) as wpool,
            tc.tile_pool(name="opool", bufs=3) as opool,
            tc.tile_pool(name="psum", bufs=NBT * NET, space="PSUM") as psum,
        ):
            ones = const.tile([1, P], mm_dt)
            nc.any.memset(ones[:], 1.0)
            zbias = const.tile([P, 1], mybir.dt.float32)
            nc.any.memset(zbias[:], 0.0)

            for c in range(CPC):
                # whole (S, B) slab for this channel: [p, k, b] in SBUF, 1 MB.
                # SWDGE ring: doesn't queue behind the W stream on SP-HWDGE.
                xsb = xpool.tile([P, KT, B], mm_dt)
                nc.gpsimd.dma_start(
                    xsb[:], xt[c, :, :].rearrange("(k p) b -> p k b", p=P)
                )
                bsb = bpool.tile([1, E], mm_dt)
                nc.gpsimd.dma_start(bsb[:], bias[c : c + 1, :])

                ps = [
                    [
                        psum.tile([P, FREE], mybir.dt.float32, name="ps")
                        for _ in range(NET)
                    ]
                    for _ in range(NBT)
                ]
                # W chunk schedule (k-tiles per DMA): ramp up at kernel start
                # so the first matmuls don't wait on a full 2 MB transfer.
                chunk_kts = [1, 1, 2, 4, 4, 4] if c == 0 else [KC] * (KT // KC)
                k = 0
                for ckt in chunk_kts:
                    wsb = wpool.tile([P, KC, E], mm_dt, name="wsb")
                    nc.sync.dma_start(
                        wsb[:, :ckt, :],
                        w[c, k * P : (k + ckt) * P, :].rearrange(
                            "(k p) e -> p k e", p=P
                        ),
                    )
                    for kk in range(ckt):
                        for bt in range(NBT):
                            lhsT = xsb[:, k, bt * P : (bt + 1) * P]
                            for et in range(NET):
                                nc.tensor.matmul(
                                    ps[bt][et][:],
                                    lhsT,
                                    wsb[:, kk, et * FREE : (et + 1) * FREE],
                                    start=(k == 0),
                                    stop=False,
                                )
                        k += 1
                # bias row: psum += ones(1,128).T @ bias(1,FREE)
                for bt in range(NBT):
                    for et in range(NET):
                        nc.tensor.matmul(
                            ps[bt][et][:],
                            ones[0:1, :],
                            bsb[0:1, et * FREE : (et + 1) * FREE],
                            start=False,
                            stop=True,
                        )
                # evict with fused ReLU on ScalarE into one (128, E) row,
                # then one 1 MB DMA out per bt
                for bt in range(NBT):
                    ot = opool.tile([P, E], mybir.dt.float32)
                    for et in range(NET):
                        nc.scalar.activation(
                            ot[:, et * FREE : (et + 1) * FREE],
                            ps[bt][et][:],
                            mybir.ActivationFunctionType.Relu,
                            bias=zbias[:],
                        )
                    # output on the ACT HWDGE ring, separate from the W stream
                    nc.scalar.dma_start(out[bt * P : (bt + 1) * P, c, :], ot[:])
    nc.compile()
    return nc


def _get_nc(mm_dtype: str):
    if mm_dtype not in _nc_cache:
        _nc_cache[mm_dtype] = _build(mm_dtype)
    return _nc_cache[mm_dtype]


def _run(x, W, b, mm_dtype=None, **spmd_kwargs):
    mm_dtype = mm_dtype or MM_DTYPE
    _, np_dt = _DTYPES[mm_dtype]
    nc = _get_nc(mm_dtype)

    in_maps = []
    for i in range(NCORES):
        c0, c1 = i * CPC, (i + 1) * CPC
        xt_i = np.ascontiguousarray(
            x[:, :, c0:c1].transpose(2, 1, 0).astype(np_dt)
        )
        w_i = np.ascontiguousarray(W[c0:c1].astype(np_dt))
        b_i = np.ascontiguousarray(b[c0:c1].astype(np_dt))
        in_maps.append({"xt": xt_i, "w": w_i, "bias": b_i})

    res = run_bass_kernel_spmd(nc, in_maps, core_ids=list(range(NCORES)), **spmd_kwargs)
    out = np.concatenate([r["out"] for r in res.results], axis=1)
    return out, res


def kernel(x: np.ndarray, W: np.ndarray, b: np.ndarray) -> np.ndarray:
    out, _ = _run(x, W, b)
    return out
